# revision 36
# baseline (speedup 1.0000x reference)
"""Causal self-attention with RoPE on 8 Trainium2 NeuronCores.

Sharding: batch x head-group. Core c handles batch b = c//2 and head group
g = c%2 (8 of 16 heads). Each core runs the full per-(batch, head-group)
pipeline on device:

  QKV^T projection -> RoPE -> causal flash-style attention -> partial
  output projection (its heads' slice of W_out rows).

The host sums the two partial projections per batch and adds b_out.

v3 pipeline (v2 measured 414us: attention phase ACT-bound at ~95% Scalar
busy while PE idled ~40%, QKV phase the reverse): QKV projection + RoPE
for chunk c+1 are emitted INTO the PE-idle slots between attention pairs
of chunk c (the PE queue is in-order, so placement in program order is
what fills the gaps).  Per-pair boundary schedule for chunk c:
  b0: div-finishB(half1, c-1) | qkv-unit(c+1, mt0) | v-unit(c+1, 0)
  b1: div-finishA(half0, c)   | proj(c-1)          | qkv(c+1, mt1) | v(..1)
  b2: div-finishB(half0, c)   | qkv(c+1, mt2)      | v(..2)
  b3: div-finishA(half1, c)   | qkv(c+1, mt3)      | v(..3)
finishA (the 4us batched reciprocal) always lands one full pair kt-loop
before the finishB that consumes it, and proj(c) one boundary after the
final normalize-mul of chunk c, so no PE-visible dependency ever stalls.

Other changes vs the 572us baseline: both heads of a pair share one
[128,1024] score PSUM tile and ONE wide exp ACTIVATE (amortizes ACT's
352-cycle fixed cost); PV PSUM is drained to SBUF right after stop=True so
the softmax divide chain never blocks bank reuse (the baseline's serialized
4us reciprocals caused 3-4us PE gaps that HAM-throttled the PE to 1.2GHz
for 58% of the runtime); reciprocals are batched 4 denominator rows ->
one DVE op; score/exp/PV all skip the fully-masked columns left of the
causal staircase; the staircase itself is a 0/1 DVE multiply on the
[128,128] window only; QKV biases on DVE (tensor_scalar_add); bf16 output
(halves the 8MB/core output DMA); small SBUF-SBUF moves issue from the
GPSIMD DMA queue so they never queue behind bulk input loads.
"""

import os
import sys

if "/opt/trn_rl_repo" not in sys.path:
    sys.path.insert(0, "/opt/trn_rl_repo")

import numpy as np
import ml_dtypes

import concourse.bass as bass
import concourse.mybir as mybir
import concourse.tile as tile

F32 = mybir.dt.float32
F32R = mybir.dt.float32r
BF16 = mybir.dt.bfloat16

B, L, D = 4, 2048, 1024
H, DH = 16, 64
NCORES = 8
G = 2                 # head groups (cores per batch)
HPC = H // G          # heads per core = 8
DQ = HPC * DH         # per-core q/k/v width = 512
PAIRS = HPC // 2      # 128-partition head pairs = 4
CHUNK = 512           # query-chunk (matmul free dim)
NCH = L // CHUNK      # 4
KT = D // 128         # 8 k-tiles over d_model
LT = L // 128         # 16 l-tiles
VW = DH + 1           # V columns per head incl. ones column = 65

LAST_RESULTS = None   # test harness reads perf fields from here


def legalize_bir_waits(bir_json: bytes) -> bytes:
    """Split multi-wait sync_infos into standalone EventSemaphore instrs.

    This container's walrus codegen accepts at most ONE sync wait per
    instruction (two for EventSemaphore), but Tile's sem assigner happily
    attaches several.  For every instruction carrying N>1 waits, keep one
    and hoist the rest onto EventSemaphore instructions inserted directly
    before it on the same engine (same block), which preserves each
    engine's program order and therefore the sync semantics.
    """
    import json as _json

    j = _json.loads(bir_json)
    uid = [0]
    for fn in j["functions"]:
        for blk in fn["blocks"]:
            out_insts = []
            for inst in blk["instructions"]:
                si = inst.get("sync_info")
                waits = (si or {}).get("on_wait") or []
                cap = 2 if inst.get("opcode") == "EventSemaphore" else 1
                if len(waits) > cap:
                    extra, keep = waits[:-cap], waits[-cap:]
                    for i in range(0, len(extra), 2):
                        uid[0] += 1
                        out_insts.append(
                            {
                                "name": f"antwaitfix-{uid[0]}",
                                "opcode": "EventSemaphore",
                                "engine": inst["engine"],
                                "ins": [],
                                "outs": [],
                                "debug": inst.get("debug", 0),
                                "sync_info": {
                                    "on_wait": extra[i : i + 2],
                                    "on_update": [],
                                },
                            }
                        )
                    si["on_wait"] = keep
                out_insts.append(inst)
            blk["instructions"] = out_insts
    return _json.dumps(j).encode()


def build_module():
    nc = bass.Bass(use_seq_codegen=True)

    xT = nc.declare_dram_parameter("xT", [D, L], BF16, isOutput=False)
    wq = nc.declare_dram_parameter("wq", [D, DQ], BF16, isOutput=False)
    wk = nc.declare_dram_parameter("wk", [D, DQ], BF16, isOutput=False)
    wv = nc.declare_dram_parameter("wv", [D, DQ], BF16, isOutput=False)
    wo = nc.declare_dram_parameter("wo", [DQ, D], BF16, isOutput=False)
    bq = nc.declare_dram_parameter("bq", [128, PAIRS], F32, isOutput=False)
    bk = nc.declare_dram_parameter("bk", [128, PAIRS], F32, isOutput=False)
    bv = nc.declare_dram_parameter("bv", [128, DQ], F32, isOutput=False)
    cosT = nc.declare_dram_parameter("cosT", [128, L], BF16, isOutput=False)
    sinT = nc.declare_dram_parameter("sinT", [128, L], BF16, isOutput=False)
    trin = nc.declare_dram_parameter("trin", [128, 128], BF16, isOutput=False)
    out = nc.declare_dram_parameter("out", [L, D], BF16, isOutput=True)

    with tile.TileContext(nc) as tc:
        with (
            tc.tile_pool(name="const", bufs=1) as cp,
            tc.tile_pool(name="acts", bufs=1) as ap,
            tc.tile_pool(name="work", bufs=2) as wp,
            tc.tile_pool(name="psum", bufs=2, space="PSUM") as ps,
        ):
            # ---- input loads, ordered so the first QKV chain can start
            # ~1.3us in: per-kt rounds of (wq, wk, xT chunk-0), then wv and
            # the rest of xT chunk by chunk.
            xT_sb = ap.tile([128, KT, L], BF16)
            wq_sb = cp.tile([128, KT, DQ], BF16)
            wk_sb = cp.tile([128, KT, DQ], BF16)
            wv_sb = cp.tile([128, KT, DQ], BF16)
            xT_r = xT.rearrange("(kt p) l -> p kt l", p=128)
            # first k-tile round fans out over three engine DMA queues so
            # the first QKV matmul's inputs land in parallel
            wq_r = wq.rearrange("(kt p) m -> p kt m", p=128)
            wk_r = wk.rearrange("(kt p) m -> p kt m", p=128)
            wv_r = wv.rearrange("(kt p) m -> p kt m", p=128)
            for kt in range(KT):
                nc.sync.dma_start(wq_sb[:, kt, :], wq_r[:, kt, :])
                (nc.gpsimd if kt < 2 else nc.sync).dma_start(
                    wk_sb[:, kt, :], wk_r[:, kt, :]
                )
                (nc.scalar if kt < 2 else nc.sync).dma_start(
                    xT_sb[:, kt, 0:CHUNK], xT_r[:, kt, 0:CHUNK]
                )
                nc.sync.dma_start(wv_sb[:, kt, :], wv_r[:, kt, :])
            bq_sb = cp.tile([128, PAIRS], F32)
            bk_sb = cp.tile([128, PAIRS], F32)
            bv_sb = cp.tile([128, DQ], F32)
            cos_sb = cp.tile([128, L], BF16)
            sin_sb = cp.tile([128, L], BF16)
            tri_sb = cp.tile([128, 128], BF16)
            nc.sync.dma_start(cos_sb[:], cosT[:])
            nc.sync.dma_start(sin_sb[:], sinT[:])
            nc.sync.dma_start(bq_sb[:], bq[:])
            nc.sync.dma_start(bk_sb[:], bk[:])
            nc.sync.dma_start(bv_sb[:], bv[:])
            nc.sync.dma_start(tri_sb[:], trin[:])
            # rest of xT streams in while chunk-0 computes; wo last (first
            # needed by proj(0) at ~55us).  These stay on the sync queue —
            # the gpsimd queue must stay clear for the latency-critical
            # RoPE-swap / den-stage moves.
            for kt in range(KT):
                nc.sync.dma_start(
                    xT_sb[:, kt, CHUNK:L], xT_r[:, kt, CHUNK:L]
                )
            wo_sb = cp.tile([128, PAIRS, D], BF16)
            for pr in range(PAIRS):
                nc.sync.dma_start(
                    wo_sb[:, pr, :], wo.rearrange("(pr p) c -> p pr c", p=128)[:, pr, :]
                )
            # bf16 ones/reciprocals: the K=1 broadcast matmul then runs in
            # bf16 (fp32r forces the PE's slow HIGH mode, ~4x the cycles);
            # denominators at bf16 cost ~0.4% relative, well inside the
            # tolerance budget.
            ones_sb = cp.tile([128, 64], BF16)
            nc.vector.memset(ones_sb[:], 1.0)

            qT_sb = ap.tile([128, PAIRS, L], BF16)
            kT_sb = ap.tile([128, PAIRS, L], BF16)
            v_sb = ap.tile([128, LT, HPC * VW], BF16)
            yT_sb = ap.tile([128, PAIRS, L], BF16)

            # ---------------- emission units ----------------
            def qkv_unit(c, mt):
                """q+k projection, bias, and RoPE for (pair mt, chunk c)."""
                q0 = c * CHUNK
                qk = ps.tile([128, 1024], F32, tag="sc", name=f"qk_{c}_{mt}")
                for half, w_sb in ((0, wq_sb), (1, wk_sb)):
                    for kt in range(KT):
                        nc.tensor.matmul(
                            qk[:, half * 512 : half * 512 + 512],
                            w_sb[:, kt, mt * 128 : (mt + 1) * 128],
                            xT_sb[:, kt, q0 : q0 + CHUNK],
                            start=(kt == 0),
                            stop=(kt == KT - 1),
                        )
                if c == 1:
                    nc.scalar.activation(
                        qT_sb[:, mt, q0 : q0 + CHUNK], qk[:, 0:512],
                        mybir.ActivationFunctionType.Identity,
                        bias=bq_sb[:, mt : mt + 1],
                    )
                    nc.scalar.activation(
                        kT_sb[:, mt, q0 : q0 + CHUNK], qk[:, 512:1024],
                        mybir.ActivationFunctionType.Identity,
                        bias=bk_sb[:, mt : mt + 1],
                    )
                else:
                    nc.vector.tensor_scalar_add(
                        qT_sb[:, mt, q0 : q0 + CHUNK], qk[:, 0:512],
                        bq_sb[:, mt : mt + 1],
                    )
                    nc.vector.tensor_scalar_add(
                        kT_sb[:, mt, q0 : q0 + CHUNK], qk[:, 512:1024],
                        bk_sb[:, mt : mt + 1],
                    )
                # RoPE in place on this (pair, chunk) slice; the rotate-half
                # partition swaps run on the idle GPSIMD engine (it has no
                # partition-base-match restriction), keeping SP/DVE free.
                for dst in (qT_sb, kT_sb):
                    t = dst[:, mt, q0 : q0 + CHUNK]
                    swp = wp.tile([128, CHUNK], BF16, tag="swp", bufs=2)
                    for i in range(4):
                        j = i ^ 1
                        nc.gpsimd.dma_start(
                            swp[i * 32 : (i + 1) * 32, :],
                            t[j * 32 : (j + 1) * 32, :],
                        )
                    nc.vector.tensor_mul(
                        swp[:], swp[:], sin_sb[:, q0 : q0 + CHUNK]
                    )
                    nc.vector.tensor_mul(t, t, cos_sb[:, q0 : q0 + CHUNK])
                    nc.vector.tensor_add(t, t, swp[:])

            def v_unit(lt):
                """V projection (+bias, ones column) for l-tile lt."""
                vps = ps.tile([128, CHUNK], F32, tag="qp", name=f"vps_{lt}")
                for kt in range(KT):
                    nc.tensor.matmul(
                        vps[:],
                        xT_sb[:, kt, lt * 128 : (lt + 1) * 128],
                        wv_sb[:, kt, :],
                        start=(kt == 0),
                        stop=(kt == KT - 1),
                    )
                vdst = v_sb[:, lt, :].rearrange("p (h c) -> p h c", c=VW)
                nc.vector.tensor_add(vdst[:, :, 0:DH], vps[:], bv_sb[:])
                nc.vector.memset(vdst[:, :, DH:VW], 1.0)

            def make_proj(c):
                def proj():
                    last = c == NCH - 1
                    if last:
                        obig = wp.tile([128, 4, 2, CHUNK], BF16, tag="obig",
                                       bufs=1)
                    for lt in range(4 * c, 4 * c + 4):
                        for cc in range(2):
                            op = ps.tile([128, CHUNK], F32, tag="qp",
                                         name=f"op_{lt}_{cc}")
                            for pr in range(PAIRS):
                                nc.tensor.matmul(
                                    op[:],
                                    yT_sb[:, pr, lt * 128 : (lt + 1) * 128],
                                    wo_sb[:, pr, cc * CHUNK : (cc + 1) * CHUNK],
                                    start=(pr == 0),
                                    stop=(pr == PAIRS - 1),
                                )
                            if last:
                                # final chunk: gather the 8 tiles in SBUF and
                                # write them with ONE descriptor -- the tail
                                # otherwise serializes 8 DMA issues at ~0.6us
                                nc.vector.tensor_copy(
                                    obig[:, lt - 4 * c, cc, :], op[:]
                                )
                            else:
                                ob = wp.tile([128, CHUNK], BF16, tag="ob",
                                             bufs=2)
                                nc.vector.tensor_copy(ob[:], op[:])
                                nc.sync.dma_start(
                                    out[
                                        lt * 128 : (lt + 1) * 128,
                                        cc * CHUNK : (cc + 1) * CHUNK,
                                    ],
                                    ob[:],
                                )
                    if last:
                        nc.sync.dma_start(
                            out[4 * c * 128 : (4 * c + 4) * 128, :].rearrange(
                                "(lt p) (cc w) -> p lt cc w", p=128, w=CHUNK
                            ),
                            obig[:],
                        )
                return proj

            # ---- pipeline prologue: chunk-0 QKV kt-OUTER across all 4
            # pairs (8 MMs per arriving xT k-tile -> full PE duty while the
            # input streams in).  mt 0/1 use the two [128,1024] sc tiles,
            # mt 2/3 split their q/k chains across the ys/qp rings — the
            # whole 8-bank PSUM is otherwise idle during the prologue.
            qk01 = [ps.tile([128, 1024], F32, tag="sc", name=f"qk0_{m}")
                    for m in (0, 1)]
            q2 = ps.tile([128, CHUNK], F32, tag="ys", name="q2_pro")
            k2 = ps.tile([128, CHUNK], F32, tag="ys", name="k2_pro")
            q3 = ps.tile([128, CHUNK], F32, tag="qp", name="q3_pro")
            k3 = ps.tile([128, CHUNK], F32, tag="qp", name="k3_pro")
            pro = {
                (0, 0): qk01[0][:, 0:512], (0, 1): qk01[0][:, 512:1024],
                (1, 0): qk01[1][:, 0:512], (1, 1): qk01[1][:, 512:1024],
                (2, 0): q2[:], (2, 1): k2[:],
                (3, 0): q3[:], (3, 1): k3[:],
            }
            for kt in range(KT):
                for mt in range(PAIRS):
                    for half, w_sb in ((0, wq_sb), (1, wk_sb)):
                        nc.tensor.matmul(
                            pro[(mt, half)],
                            w_sb[:, kt, mt * 128 : (mt + 1) * 128],
                            xT_sb[:, kt, 0:CHUNK],
                            start=(kt == 0),
                            stop=(kt == KT - 1),
                        )
            for lt in range(4):
                v_unit(lt)
            for mt in range(PAIRS):
                nc.scalar.activation(
                    qT_sb[:, mt, 0:CHUNK], pro[(mt, 0)],
                    mybir.ActivationFunctionType.Identity,
                    bias=bq_sb[:, mt : mt + 1],
                )
                nc.scalar.activation(
                    kT_sb[:, mt, 0:CHUNK], pro[(mt, 1)],
                    mybir.ActivationFunctionType.Identity,
                    bias=bk_sb[:, mt : mt + 1],
                )
                for dst in (qT_sb, kT_sb):
                    t = dst[:, mt, 0:CHUNK]
                    swp = wp.tile([128, CHUNK], BF16, tag="swp", bufs=2)
                    for i in range(4):
                        j = i ^ 1
                        nc.gpsimd.dma_start(
                            swp[i * 32 : (i + 1) * 32, :],
                            t[j * 32 : (j + 1) * 32, :],
                        )
                    nc.vector.tensor_mul(swp[:], swp[:], sin_sb[:, 0:CHUNK])
                    nc.vector.tensor_mul(t, t, cos_sb[:, 0:CHUNK])
                    nc.vector.tensor_add(t, t, swp[:])

            # ---- attention pipeline
            pending = []           # staggered div-finish / proj closures

            def drain_pending(n):
                for _ in range(min(n, len(pending))):
                    pending.pop(0)()

            for c in range(NCH):
                q0 = c * CHUNK
                n_lk = 4 * (c + 1)
                half_state = {}
                for pr in range(PAIRS):
                    ys = [
                        ps.tile([128, CHUNK], F32, tag="ys",
                                name=f"ys_{c}_{pr}_{hh}")
                        for hh in range(2)
                    ]
                    for kt in range(n_lk):
                        k0 = kt * 128
                        off = max(0, k0 - q0)
                        sps = ps.tile([128, 1024], F32, tag="sc",
                                      name=f"sps_{c}_{pr}_{kt}")
                        diag = k0 >= q0
                        for hh in range(2):
                            # columns left of the staircase are fully masked
                            # and never read (exp and PV skip them too), so
                            # the score matmul starts at `off`.
                            nc.tensor.matmul(
                                sps[:, hh * 512 + off : hh * 512 + 512],
                                kT_sb[hh * 64 : (hh + 1) * 64, pr, k0 : k0 + 128],
                                qT_sb[hh * 64 : (hh + 1) * 64, pr,
                                      q0 + off : q0 + CHUNK],
                                start=True,
                                stop=True,
                                tile_position=(hh * 64, 0),
                            )
                        ex = wp.tile([128, 1024], BF16, tag="ex", bufs=4)
                        if off:
                            # skip the fully-masked columns left of the
                            # staircase (strided 2-window AP)
                            nc.scalar.activation(
                                ex.rearrange("p (h w) -> p h w", w=512)[:, :, off:],
                                sps.rearrange("p (h w) -> p h w", w=512)[:, :, off:],
                                mybir.ActivationFunctionType.Exp,
                                scale=float(1.0 / np.sqrt(DH)),
                            )
                        else:
                            nc.scalar.activation(
                                ex[:], sps[:], mybir.ActivationFunctionType.Exp,
                                scale=float(1.0 / np.sqrt(DH)),
                            )
                        if diag:
                            for hh in range(2):
                                nc.vector.tensor_mul(
                                    ex[:, hh * 512 + off : hh * 512 + off + 128],
                                    ex[:, hh * 512 + off : hh * 512 + off + 128],
                                    tri_sb[:],
                                )
                        for hh in range(2):
                            h = 2 * pr + hh
                            nc.tensor.matmul(
                                ys[hh][0:VW, off:CHUNK],
                                v_sb[:, kt, h * VW : (h + 1) * VW],
                                ex[:, hh * 512 + off : (hh + 1) * 512],
                                start=(kt == 0),
                                stop=(kt == n_lk - 1),
                            )
                    # drain PV psum to SBUF fast (frees banks; PE rolls on)
                    if pr % 2 == 0:
                        stage = wp.tile([128, 128], F32, tag="stage", bufs=2,
                                        name=f"stage_{c}_{pr // 2}")
                        half_state = {"stage": stage, "tiles": []}
                    for hh in range(2):
                        ya = wp.tile([65, CHUNK], F32, tag="ya", bufs=10,
                                     name=f"ya_{c}_{pr}_{hh}")
                        nc.vector.tensor_copy(ya[:], ys[hh][0:VW, :])
                        idx = (pr % 2) * 2 + hh
                        # pack the [1,512] denominator row as [4,128] across
                        # partitions so the reciprocal runs 4 lanes wide
                        nc.gpsimd.dma_start(
                            half_state["stage"][32 * idx : 32 * idx + 4, :],
                            ya[64:65, :].rearrange("p (a j) -> p a j", a=4),
                        )
                        half_state["tiles"].append((pr, hh, ya, idx))
                    if pr % 2 == 1:
                        st = half_state
                        stash = {}

                        def finishA(st=st, stash=stash):
                            stage_r = wp.tile([128, 128], BF16, tag="str",
                                              bufs=2)
                            with nc.allow_low_precision(reason="bf16 recip"):
                                nc.vector.reciprocal(stage_r[:], st["stage"][:])
                            den4 = wp.tile([128, CHUNK], BF16, tag="den4",
                                           bufs=2)
                            for _, _, _, idx in st["tiles"]:
                                nc.gpsimd.dma_start(
                                    den4[32 * idx : 32 * idx + 1, :],
                                    stage_r[32 * idx : 32 * idx + 4, :],
                                )
                            stash["den4"] = den4

                        def finishB(st=st, stash=stash, q0=q0):
                            den4 = stash["den4"]
                            for pr2, hh, ya, idx in st["tiles"]:
                                bc = ps.tile([128, CHUNK], F32, tag="qp",
                                             name=f"bc_{q0}_{pr2}_{hh}")
                                nc.tensor.matmul(
                                    bc[0:64, :],
                                    ones_sb[32 * idx : 32 * idx + 1, :],
                                    den4[32 * idx : 32 * idx + 1, :],
                                    start=True,
                                    stop=True,
                                    tile_position=(32 * idx, 0),
                                )
                                if hh == 0:
                                    nc.vector.tensor_mul(
                                        yT_sb[0:64, pr2, q0 : q0 + CHUNK],
                                        ya[0:64, :],
                                        bc[0:64, :],
                                    )
                                else:
                                    # elementwise out/in partition bases must
                                    # match; base-0 tmp + DMA moves to 64:128.
                                    yt = wp.tile([64, CHUNK], BF16, tag="yt",
                                                 bufs=2)
                                    nc.vector.tensor_mul(
                                        yt[:], ya[0:64, :], bc[0:64, :]
                                    )
                                    nc.gpsimd.dma_start(
                                        yT_sb[64:128, pr2, q0 : q0 + CHUNK],
                                        yt[:],
                                    )

                        pending.append(finishA)
                        pending.append(finishB)
                    # ---- pair boundary: next chunk's QKV/V first (their DVE
                    # bias/RoPE ops gate a later pair's score matmuls, so
                    # they must beat the div-chain's reciprocal into the DVE
                    # queue), then the staggered div/proj items.  Drains must
                    # total 5 per chunk (2 finishes x2 + proj) or the queue
                    # drifts and the tail serializes; the double-drain sits
                    # at p1 (proj(c-1) then finishA) where both items'
                    # dependencies are a full pair-loop old.
                    if c + 1 < NCH:
                        qkv_unit(c + 1, pr)
                        v_unit(4 * (c + 1) + pr)
                    drain_pending(2 if (pr == 1 and c > 0) else 1)

                pending.append(make_proj(c))
            drain_pending(len(pending))
    return nc


def _rope_tables():
    inv_freq = (1.0 / (10000.0 ** (np.arange(0, DH, 2, dtype=np.float32) / DH))).astype(
        np.float32
    )
    t = np.arange(L, dtype=np.float32)
    freqs = np.einsum("l,d->ld", t, inv_freq).astype(np.float32)  # (L, 32)
    emb = np.concatenate([freqs, freqs], axis=-1)                 # (L, 64)
    cos = np.cos(emb).astype(np.float32)
    sin = np.sin(emb).astype(np.float32)
    cosT = cos.T                                   # (64, L)
    sinT = sin.T.copy()
    sinT[0:32] = -sinT[0:32]                       # fold rotate_half sign
    cos128 = np.tile(cosT, (2, 1))                 # (128, L)
    sin128 = np.tile(sinT, (2, 1))
    return cos128, sin128


def _tri_neg():
    # trin[p, t] = 0 where key-partition p is masked for query-col t of the
    # staircase window (p > t), else 1 (multiplicative mask on exp scores).
    p = np.arange(128)[:, None]
    t = np.arange(128)[None, :]
    return (p <= t).astype(np.float32)


def _bf16(a):
    return np.asarray(a, dtype=np.float32).astype(ml_dtypes.bfloat16)


_COMPILED = None


def _ensure_ntff_hook():
    """This image's antenv lacks axon_hooks; trace=True would crash on the
    import inside run_bass_kernel_spmd. Inject the module and register the
    ctypes NTFF hook so tracing works; silently skip if anything is off."""
    import types

    if "antenv.axon_hooks" in sys.modules:
        return
    try:
        mod = types.ModuleType("antenv.axon_hooks")
        store = [None]
        mod.set_axon_ntff_profile_hook = lambda h: store.__setitem__(0, h)
        mod.get_axon_ntff_profile_hook = lambda: store[0]
        from trn_agent_boot.trn_boot import _ntff_profile_via_ctypes

        mod.set_axon_ntff_profile_hook(
            _ntff_profile_via_ctypes("/opt/axon/libaxon_pjrt.so")
        )
        sys.modules["antenv.axon_hooks"] = mod
    except Exception:
        pass


def kernel(x, pad_mask, W_qkv, b_qkv, W_out, b_out):
    global LAST_RESULTS, _COMPILED
    if os.environ.get("BASS_TRACE"):
        _ensure_ntff_hook()
    from concourse.bass_utils import run_bass_kernel_spmd

    x = np.asarray(x, dtype=np.float32)
    W_qkv = np.asarray(W_qkv, dtype=np.float32)
    b_qkv = np.asarray(b_qkv, dtype=np.float32)
    W_out = np.asarray(W_out, dtype=np.float32)
    b_out = np.asarray(b_out, dtype=np.float32)

    cos128, sin128 = _rope_tables()

    in_maps = []
    for core in range(NCORES):
        b, g = core // G, core % G
        sl = slice(g * DQ, (g + 1) * DQ)
        wqv = W_qkv[:, 0 * D : 1 * D][:, sl]
        wkv = W_qkv[:, 1 * D : 2 * D][:, sl]
        wvv = W_qkv[:, 2 * D : 3 * D][:, sl]
        bqv = b_qkv[0 * D : 1 * D][sl]
        bkv = b_qkv[1 * D : 2 * D][sl]
        bvv = b_qkv[2 * D : 3 * D][sl]
        in_maps.append(
            {
                "xT": _bf16(x[b].T),
                "wq": _bf16(wqv),
                "wk": _bf16(wkv),
                "wv": _bf16(wvv),
                "wo": _bf16(W_out[sl, :]),
                "bq": np.ascontiguousarray(bqv.reshape(PAIRS, 128).T),
                "bk": np.ascontiguousarray(bkv.reshape(PAIRS, 128).T),
                "bv": np.tile(bvv[None, :], (128, 1)).astype(np.float32),
                "cosT": _bf16(cos128),
                "sinT": _bf16(sin128),
                "trin": _bf16(_tri_neg()),
            }
        )

    if _COMPILED is None:
        nc = build_module()
        fixed = legalize_bir_waits(nc.to_json_bytes())
        nc.to_json_bytes = lambda: fixed  # bass2jax ships this BIR to walrus
        _COMPILED = nc
    nc = _COMPILED

    res = run_bass_kernel_spmd(
        nc,
        in_maps,
        core_ids=list(range(NCORES)),
        trace=bool(os.environ.get("BASS_TRACE")),
    )
    LAST_RESULTS = res

    out = np.zeros((B, L, D), dtype=np.float32)
    for core in range(NCORES):
        out[core // G] += np.asarray(res.results[core]["out"], dtype=np.float32)
    out += b_out[None, None, :]
    return out


# revision 37
# speedup vs baseline: 1.0050x; 1.0050x over previous
"""Causal self-attention with RoPE on 8 Trainium2 NeuronCores.

Sharding: batch x head-group. Core c handles batch b = c//2 and head group
g = c%2 (8 of 16 heads). Each core runs the full per-(batch, head-group)
pipeline on device:

  QKV^T projection -> RoPE -> causal flash-style attention -> partial
  output projection (its heads' slice of W_out rows).

The host sums the two partial projections per batch and adds b_out.

v3 pipeline (v2 measured 414us: attention phase ACT-bound at ~95% Scalar
busy while PE idled ~40%, QKV phase the reverse): QKV projection + RoPE
for chunk c+1 are emitted INTO the PE-idle slots between attention pairs
of chunk c (the PE queue is in-order, so placement in program order is
what fills the gaps).  Per-pair boundary schedule for chunk c:
  b0: div-finishB(half1, c-1) | qkv-unit(c+1, mt0) | v-unit(c+1, 0)
  b1: div-finishA(half0, c)   | proj(c-1)          | qkv(c+1, mt1) | v(..1)
  b2: div-finishB(half0, c)   | qkv(c+1, mt2)      | v(..2)
  b3: div-finishA(half1, c)   | qkv(c+1, mt3)      | v(..3)
finishA (the 4us batched reciprocal) always lands one full pair kt-loop
before the finishB that consumes it, and proj(c) one boundary after the
final normalize-mul of chunk c, so no PE-visible dependency ever stalls.

Other changes vs the 572us baseline: both heads of a pair share one
[128,1024] score PSUM tile and ONE wide exp ACTIVATE (amortizes ACT's
352-cycle fixed cost); PV PSUM is drained to SBUF right after stop=True so
the softmax divide chain never blocks bank reuse (the baseline's serialized
4us reciprocals caused 3-4us PE gaps that HAM-throttled the PE to 1.2GHz
for 58% of the runtime); reciprocals are batched 4 denominator rows ->
one DVE op; score/exp/PV all skip the fully-masked columns left of the
causal staircase; the staircase itself is a 0/1 DVE multiply on the
[128,128] window only; QKV biases on DVE (tensor_scalar_add); bf16 output
(halves the 8MB/core output DMA); small SBUF-SBUF moves issue from the
GPSIMD DMA queue so they never queue behind bulk input loads.
"""

import os
import sys

if "/opt/trn_rl_repo" not in sys.path:
    sys.path.insert(0, "/opt/trn_rl_repo")

import numpy as np
import ml_dtypes

import concourse.bass as bass
import concourse.mybir as mybir
import concourse.tile as tile

F32 = mybir.dt.float32
F32R = mybir.dt.float32r
BF16 = mybir.dt.bfloat16

B, L, D = 4, 2048, 1024
H, DH = 16, 64
NCORES = 8
G = 2                 # head groups (cores per batch)
HPC = H // G          # heads per core = 8
DQ = HPC * DH         # per-core q/k/v width = 512
PAIRS = HPC // 2      # 128-partition head pairs = 4
CHUNK = 512           # query-chunk (matmul free dim)
NCH = L // CHUNK      # 4
KT = D // 128         # 8 k-tiles over d_model
LT = L // 128         # 16 l-tiles
VW = DH + 1           # V columns per head incl. ones column = 65

LAST_RESULTS = None   # test harness reads perf fields from here


def legalize_bir_waits(bir_json: bytes) -> bytes:
    """Split multi-wait sync_infos into standalone EventSemaphore instrs.

    This container's walrus codegen accepts at most ONE sync wait per
    instruction (two for EventSemaphore), but Tile's sem assigner happily
    attaches several.  For every instruction carrying N>1 waits, keep one
    and hoist the rest onto EventSemaphore instructions inserted directly
    before it on the same engine (same block), which preserves each
    engine's program order and therefore the sync semantics.
    """
    import json as _json

    j = _json.loads(bir_json)
    uid = [0]
    for fn in j["functions"]:
        for blk in fn["blocks"]:
            out_insts = []
            for inst in blk["instructions"]:
                si = inst.get("sync_info")
                waits = (si or {}).get("on_wait") or []
                cap = 2 if inst.get("opcode") == "EventSemaphore" else 1
                if len(waits) > cap:
                    extra, keep = waits[:-cap], waits[-cap:]
                    for i in range(0, len(extra), 2):
                        uid[0] += 1
                        out_insts.append(
                            {
                                "name": f"antwaitfix-{uid[0]}",
                                "opcode": "EventSemaphore",
                                "engine": inst["engine"],
                                "ins": [],
                                "outs": [],
                                "debug": inst.get("debug", 0),
                                "sync_info": {
                                    "on_wait": extra[i : i + 2],
                                    "on_update": [],
                                },
                            }
                        )
                    si["on_wait"] = keep
                out_insts.append(inst)
            blk["instructions"] = out_insts
    return _json.dumps(j).encode()


def build_module():
    nc = bass.Bass(use_seq_codegen=True)

    xT = nc.declare_dram_parameter("xT", [D, L], BF16, isOutput=False)
    wq = nc.declare_dram_parameter("wq", [D, DQ], BF16, isOutput=False)
    wk = nc.declare_dram_parameter("wk", [D, DQ], BF16, isOutput=False)
    wv = nc.declare_dram_parameter("wv", [D, DQ], BF16, isOutput=False)
    wo = nc.declare_dram_parameter("wo", [DQ, D], BF16, isOutput=False)
    bq = nc.declare_dram_parameter("bq", [128, PAIRS], F32, isOutput=False)
    bk = nc.declare_dram_parameter("bk", [128, PAIRS], F32, isOutput=False)
    bv = nc.declare_dram_parameter("bv", [128, DQ], F32, isOutput=False)
    cosT = nc.declare_dram_parameter("cosT", [128, L], BF16, isOutput=False)
    sinT = nc.declare_dram_parameter("sinT", [128, L], BF16, isOutput=False)
    trin = nc.declare_dram_parameter("trin", [128, 128], BF16, isOutput=False)
    out = nc.declare_dram_parameter("out", [L, D], BF16, isOutput=True)

    with tile.TileContext(nc) as tc:
        with (
            tc.tile_pool(name="const", bufs=1) as cp,
            tc.tile_pool(name="acts", bufs=1) as ap,
            tc.tile_pool(name="work", bufs=2) as wp,
            tc.tile_pool(name="psum", bufs=2, space="PSUM") as ps,
        ):
            # ---- input loads, ordered so the first QKV chain can start
            # ~1.3us in: per-kt rounds of (wq, wk, xT chunk-0), then wv and
            # the rest of xT chunk by chunk.
            xT_sb = ap.tile([128, KT, L], BF16)
            wq_sb = cp.tile([128, KT, DQ], BF16)
            wk_sb = cp.tile([128, KT, DQ], BF16)
            wv_sb = cp.tile([128, KT, DQ], BF16)
            xT_r = xT.rearrange("(kt p) l -> p kt l", p=128)
            # first k-tile round fans out over three engine DMA queues so
            # the first QKV matmul's inputs land in parallel
            wq_r = wq.rearrange("(kt p) m -> p kt m", p=128)
            wk_r = wk.rearrange("(kt p) m -> p kt m", p=128)
            wv_r = wv.rearrange("(kt p) m -> p kt m", p=128)
            for kt in range(KT):
                nc.sync.dma_start(wq_sb[:, kt, :], wq_r[:, kt, :])
                (nc.gpsimd if kt < 2 else nc.sync).dma_start(
                    wk_sb[:, kt, :], wk_r[:, kt, :]
                )
                (nc.scalar if kt < 2 else nc.sync).dma_start(
                    xT_sb[:, kt, 0:CHUNK], xT_r[:, kt, 0:CHUNK]
                )
                nc.sync.dma_start(wv_sb[:, kt, :], wv_r[:, kt, :])
            bq_sb = cp.tile([128, PAIRS], F32)
            bk_sb = cp.tile([128, PAIRS], F32)
            bv_sb = cp.tile([128, DQ], F32)
            cos_sb = cp.tile([128, L], BF16)
            sin_sb = cp.tile([128, L], BF16)
            tri_sb = cp.tile([128, 128], BF16)
            nc.sync.dma_start(cos_sb[:], cosT[:])
            nc.sync.dma_start(sin_sb[:], sinT[:])
            nc.sync.dma_start(bq_sb[:], bq[:])
            nc.sync.dma_start(bk_sb[:], bk[:])
            nc.sync.dma_start(bv_sb[:], bv[:])
            nc.sync.dma_start(tri_sb[:], trin[:])
            # rest of xT streams in while chunk-0 computes; wo last (first
            # needed by proj(0) at ~55us).  These stay on the sync queue —
            # the gpsimd queue must stay clear for the latency-critical
            # RoPE-swap / den-stage moves.
            for kt in range(KT):
                nc.sync.dma_start(
                    xT_sb[:, kt, CHUNK:L], xT_r[:, kt, CHUNK:L]
                )
            wo_sb = cp.tile([128, PAIRS, D], BF16)
            for pr in range(PAIRS):
                nc.sync.dma_start(
                    wo_sb[:, pr, :], wo.rearrange("(pr p) c -> p pr c", p=128)[:, pr, :]
                )
            # bf16 ones/reciprocals: the K=1 broadcast matmul then runs in
            # bf16 (fp32r forces the PE's slow HIGH mode, ~4x the cycles);
            # denominators at bf16 cost ~0.4% relative, well inside the
            # tolerance budget.
            ones_sb = cp.tile([128, 64], BF16)
            nc.vector.memset(ones_sb[:], 1.0)

            qT_sb = ap.tile([128, PAIRS, L], BF16)
            kT_sb = ap.tile([128, PAIRS, L], BF16)
            v_sb = ap.tile([128, LT, HPC * VW], BF16)
            yT_sb = ap.tile([128, PAIRS, L], BF16)

            # ---------------- emission units ----------------
            def qkv_unit(c, mt):
                """q+k projection, bias, and RoPE for (pair mt, chunk c)."""
                q0 = c * CHUNK
                qk = ps.tile([128, 1024], F32, tag="sc", name=f"qk_{c}_{mt}")
                for half, w_sb in ((0, wq_sb), (1, wk_sb)):
                    for kt in range(KT):
                        nc.tensor.matmul(
                            qk[:, half * 512 : half * 512 + 512],
                            w_sb[:, kt, mt * 128 : (mt + 1) * 128],
                            xT_sb[:, kt, q0 : q0 + CHUNK],
                            start=(kt == 0),
                            stop=(kt == KT - 1),
                        )
                nc.vector.tensor_scalar_add(
                    qT_sb[:, mt, q0 : q0 + CHUNK], qk[:, 0:512],
                    bq_sb[:, mt : mt + 1],
                )
                nc.vector.tensor_scalar_add(
                    kT_sb[:, mt, q0 : q0 + CHUNK], qk[:, 512:1024],
                    bk_sb[:, mt : mt + 1],
                )
                # RoPE in place on this (pair, chunk) slice; the rotate-half
                # partition swaps run on the idle GPSIMD engine (it has no
                # partition-base-match restriction), keeping SP/DVE free.
                for dst in (qT_sb, kT_sb):
                    t = dst[:, mt, q0 : q0 + CHUNK]
                    swp = wp.tile([128, CHUNK], BF16, tag="swp", bufs=2)
                    for i in range(4):
                        j = i ^ 1
                        nc.gpsimd.dma_start(
                            swp[i * 32 : (i + 1) * 32, :],
                            t[j * 32 : (j + 1) * 32, :],
                        )
                    nc.vector.tensor_mul(
                        swp[:], swp[:], sin_sb[:, q0 : q0 + CHUNK]
                    )
                    nc.vector.tensor_mul(t, t, cos_sb[:, q0 : q0 + CHUNK])
                    nc.vector.tensor_add(t, t, swp[:])

            def v_unit(lt):
                """V projection (+bias, ones column) for l-tile lt."""
                vps = ps.tile([128, CHUNK], F32, tag="qp", name=f"vps_{lt}")
                for kt in range(KT):
                    nc.tensor.matmul(
                        vps[:],
                        xT_sb[:, kt, lt * 128 : (lt + 1) * 128],
                        wv_sb[:, kt, :],
                        start=(kt == 0),
                        stop=(kt == KT - 1),
                    )
                vdst = v_sb[:, lt, :].rearrange("p (h c) -> p h c", c=VW)
                nc.vector.tensor_add(vdst[:, :, 0:DH], vps[:], bv_sb[:])
                nc.vector.memset(vdst[:, :, DH:VW], 1.0)

            def make_proj(c):
                def proj():
                    for lt in range(4 * c, 4 * c + 4):
                        for cc in range(2):
                            op = ps.tile([128, CHUNK], F32, tag="qp",
                                         name=f"op_{lt}_{cc}")
                            for pr in range(PAIRS):
                                nc.tensor.matmul(
                                    op[:],
                                    yT_sb[:, pr, lt * 128 : (lt + 1) * 128],
                                    wo_sb[:, pr, cc * CHUNK : (cc + 1) * CHUNK],
                                    start=(pr == 0),
                                    stop=(pr == PAIRS - 1),
                                )
                            ob = wp.tile([128, CHUNK], BF16, tag="ob", bufs=2)
                            nc.vector.tensor_copy(ob[:], op[:])
                            nc.sync.dma_start(
                                out[
                                    lt * 128 : (lt + 1) * 128,
                                    cc * CHUNK : (cc + 1) * CHUNK,
                                ],
                                ob[:],
                            )
                return proj

            # ---- pipeline prologue: chunk-0 QKV kt-OUTER across all 4
            # pairs (8 MMs per arriving xT k-tile -> full PE duty while the
            # input streams in).  mt 0/1 use the two [128,1024] sc tiles,
            # mt 2/3 split their q/k chains across the ys/qp rings — the
            # whole 8-bank PSUM is otherwise idle during the prologue.
            qk01 = [ps.tile([128, 1024], F32, tag="sc", name=f"qk0_{m}")
                    for m in (0, 1)]
            q2 = ps.tile([128, CHUNK], F32, tag="ys", name="q2_pro")
            k2 = ps.tile([128, CHUNK], F32, tag="ys", name="k2_pro")
            q3 = ps.tile([128, CHUNK], F32, tag="qp", name="q3_pro")
            k3 = ps.tile([128, CHUNK], F32, tag="qp", name="k3_pro")
            pro = {
                (0, 0): qk01[0][:, 0:512], (0, 1): qk01[0][:, 512:1024],
                (1, 0): qk01[1][:, 0:512], (1, 1): qk01[1][:, 512:1024],
                (2, 0): q2[:], (2, 1): k2[:],
                (3, 0): q3[:], (3, 1): k3[:],
            }
            for kt in range(KT):
                for mt in range(PAIRS):
                    for half, w_sb in ((0, wq_sb), (1, wk_sb)):
                        nc.tensor.matmul(
                            pro[(mt, half)],
                            w_sb[:, kt, mt * 128 : (mt + 1) * 128],
                            xT_sb[:, kt, 0:CHUNK],
                            start=(kt == 0),
                            stop=(kt == KT - 1),
                        )
            for lt in range(4):
                v_unit(lt)
            for mt in range(PAIRS):
                nc.vector.tensor_scalar_add(
                    qT_sb[:, mt, 0:CHUNK], pro[(mt, 0)], bq_sb[:, mt : mt + 1]
                )
                nc.vector.tensor_scalar_add(
                    kT_sb[:, mt, 0:CHUNK], pro[(mt, 1)], bk_sb[:, mt : mt + 1]
                )
                for dst in (qT_sb, kT_sb):
                    t = dst[:, mt, 0:CHUNK]
                    swp = wp.tile([128, CHUNK], BF16, tag="swp", bufs=2)
                    for i in range(4):
                        j = i ^ 1
                        nc.gpsimd.dma_start(
                            swp[i * 32 : (i + 1) * 32, :],
                            t[j * 32 : (j + 1) * 32, :],
                        )
                    nc.vector.tensor_mul(swp[:], swp[:], sin_sb[:, 0:CHUNK])
                    nc.vector.tensor_mul(t, t, cos_sb[:, 0:CHUNK])
                    nc.vector.tensor_add(t, t, swp[:])

            # ---- attention pipeline
            pending = []           # staggered div-finish / proj closures

            def drain_pending(n):
                for _ in range(min(n, len(pending))):
                    pending.pop(0)()

            for c in range(NCH):
                q0 = c * CHUNK
                n_lk = 4 * (c + 1)
                half_state = {}
                for pr in range(PAIRS):
                    ys = [
                        ps.tile([128, CHUNK], F32, tag="ys",
                                name=f"ys_{c}_{pr}_{hh}")
                        for hh in range(2)
                    ]
                    for kt in range(n_lk):
                        k0 = kt * 128
                        off = max(0, k0 - q0)
                        sps = ps.tile([128, 1024], F32, tag="sc",
                                      name=f"sps_{c}_{pr}_{kt}")
                        diag = k0 >= q0
                        for hh in range(2):
                            # columns left of the staircase are fully masked
                            # and never read (exp and PV skip them too), so
                            # the score matmul starts at `off`.
                            nc.tensor.matmul(
                                sps[:, hh * 512 + off : hh * 512 + 512],
                                kT_sb[hh * 64 : (hh + 1) * 64, pr, k0 : k0 + 128],
                                qT_sb[hh * 64 : (hh + 1) * 64, pr,
                                      q0 + off : q0 + CHUNK],
                                start=True,
                                stop=True,
                                tile_position=(hh * 64, 0),
                            )
                        ex = wp.tile([128, 1024], BF16, tag="ex", bufs=4)
                        if off:
                            # skip the fully-masked columns left of the
                            # staircase (strided 2-window AP)
                            nc.scalar.activation(
                                ex.rearrange("p (h w) -> p h w", w=512)[:, :, off:],
                                sps.rearrange("p (h w) -> p h w", w=512)[:, :, off:],
                                mybir.ActivationFunctionType.Exp,
                                scale=float(1.0 / np.sqrt(DH)),
                            )
                        else:
                            nc.scalar.activation(
                                ex[:], sps[:], mybir.ActivationFunctionType.Exp,
                                scale=float(1.0 / np.sqrt(DH)),
                            )
                        if diag:
                            for hh in range(2):
                                nc.vector.tensor_mul(
                                    ex[:, hh * 512 + off : hh * 512 + off + 128],
                                    ex[:, hh * 512 + off : hh * 512 + off + 128],
                                    tri_sb[:],
                                )
                        for hh in range(2):
                            h = 2 * pr + hh
                            nc.tensor.matmul(
                                ys[hh][0:VW, off:CHUNK],
                                v_sb[:, kt, h * VW : (h + 1) * VW],
                                ex[:, hh * 512 + off : (hh + 1) * 512],
                                start=(kt == 0),
                                stop=(kt == n_lk - 1),
                            )
                    # drain PV psum to SBUF fast (frees banks; PE rolls on)
                    if pr % 2 == 0:
                        stage = wp.tile([128, 128], F32, tag="stage", bufs=2,
                                        name=f"stage_{c}_{pr // 2}")
                        half_state = {"stage": stage, "tiles": []}
                    for hh in range(2):
                        ya = wp.tile([65, CHUNK], F32, tag="ya", bufs=10,
                                     name=f"ya_{c}_{pr}_{hh}")
                        nc.vector.tensor_copy(ya[:], ys[hh][0:VW, :])
                        idx = (pr % 2) * 2 + hh
                        # pack the [1,512] denominator row as [4,128] across
                        # partitions so the reciprocal runs 4 lanes wide
                        nc.gpsimd.dma_start(
                            half_state["stage"][32 * idx : 32 * idx + 4, :],
                            ya[64:65, :].rearrange("p (a j) -> p a j", a=4),
                        )
                        half_state["tiles"].append((pr, hh, ya, idx))
                    if pr % 2 == 1:
                        st = half_state
                        stash = {}

                        def finishA(st=st, stash=stash):
                            stage_r = wp.tile([128, 128], BF16, tag="str",
                                              bufs=2)
                            with nc.allow_low_precision(reason="bf16 recip"):
                                nc.vector.reciprocal(stage_r[:], st["stage"][:])
                            den4 = wp.tile([128, CHUNK], BF16, tag="den4",
                                           bufs=2)
                            for _, _, _, idx in st["tiles"]:
                                nc.gpsimd.dma_start(
                                    den4[32 * idx : 32 * idx + 1, :],
                                    stage_r[32 * idx : 32 * idx + 4, :],
                                )
                            stash["den4"] = den4

                        def finishB(st=st, stash=stash, q0=q0):
                            den4 = stash["den4"]
                            for pr2, hh, ya, idx in st["tiles"]:
                                bc = ps.tile([128, CHUNK], F32, tag="qp",
                                             name=f"bc_{q0}_{pr2}_{hh}")
                                nc.tensor.matmul(
                                    bc[0:64, :],
                                    ones_sb[32 * idx : 32 * idx + 1, :],
                                    den4[32 * idx : 32 * idx + 1, :],
                                    start=True,
                                    stop=True,
                                    tile_position=(32 * idx, 0),
                                )
                                if hh == 0:
                                    nc.vector.tensor_mul(
                                        yT_sb[0:64, pr2, q0 : q0 + CHUNK],
                                        ya[0:64, :],
                                        bc[0:64, :],
                                    )
                                else:
                                    # elementwise out/in partition bases must
                                    # match; base-0 tmp + DMA moves to 64:128.
                                    yt = wp.tile([64, CHUNK], BF16, tag="yt",
                                                 bufs=2)
                                    nc.vector.tensor_mul(
                                        yt[:], ya[0:64, :], bc[0:64, :]
                                    )
                                    nc.gpsimd.dma_start(
                                        yT_sb[64:128, pr2, q0 : q0 + CHUNK],
                                        yt[:],
                                    )

                        pending.append(finishA)
                        pending.append(finishB)
                    # ---- pair boundary: next chunk's QKV/V first (their DVE
                    # bias/RoPE ops gate a later pair's score matmuls, so
                    # they must beat the div-chain's reciprocal into the DVE
                    # queue), then the staggered div/proj items.  Drains must
                    # total 5 per chunk (2 finishes x2 + proj) or the queue
                    # drifts and the tail serializes; the double-drain sits
                    # at p1 (proj(c-1) then finishA) where both items'
                    # dependencies are a full pair-loop old.
                    if c + 1 < NCH:
                        qkv_unit(c + 1, pr)
                        v_unit(4 * (c + 1) + pr)
                    drain_pending(2 if (pr == 1 and c > 0) else 1)

                pending.append(make_proj(c))
            drain_pending(len(pending))
    return nc


def _rope_tables():
    inv_freq = (1.0 / (10000.0 ** (np.arange(0, DH, 2, dtype=np.float32) / DH))).astype(
        np.float32
    )
    t = np.arange(L, dtype=np.float32)
    freqs = np.einsum("l,d->ld", t, inv_freq).astype(np.float32)  # (L, 32)
    emb = np.concatenate([freqs, freqs], axis=-1)                 # (L, 64)
    cos = np.cos(emb).astype(np.float32)
    sin = np.sin(emb).astype(np.float32)
    cosT = cos.T                                   # (64, L)
    sinT = sin.T.copy()
    sinT[0:32] = -sinT[0:32]                       # fold rotate_half sign
    cos128 = np.tile(cosT, (2, 1))                 # (128, L)
    sin128 = np.tile(sinT, (2, 1))
    return cos128, sin128


def _tri_neg():
    # trin[p, t] = 0 where key-partition p is masked for query-col t of the
    # staircase window (p > t), else 1 (multiplicative mask on exp scores).
    p = np.arange(128)[:, None]
    t = np.arange(128)[None, :]
    return (p <= t).astype(np.float32)


def _bf16(a):
    return np.asarray(a, dtype=np.float32).astype(ml_dtypes.bfloat16)


_COMPILED = None


def _ensure_ntff_hook():
    """This image's antenv lacks axon_hooks; trace=True would crash on the
    import inside run_bass_kernel_spmd. Inject the module and register the
    ctypes NTFF hook so tracing works; silently skip if anything is off."""
    import types

    if "antenv.axon_hooks" in sys.modules:
        return
    try:
        mod = types.ModuleType("antenv.axon_hooks")
        store = [None]
        mod.set_axon_ntff_profile_hook = lambda h: store.__setitem__(0, h)
        mod.get_axon_ntff_profile_hook = lambda: store[0]
        from trn_agent_boot.trn_boot import _ntff_profile_via_ctypes

        mod.set_axon_ntff_profile_hook(
            _ntff_profile_via_ctypes("/opt/axon/libaxon_pjrt.so")
        )
        sys.modules["antenv.axon_hooks"] = mod
    except Exception:
        pass


def kernel(x, pad_mask, W_qkv, b_qkv, W_out, b_out):
    global LAST_RESULTS, _COMPILED
    if os.environ.get("BASS_TRACE"):
        _ensure_ntff_hook()
    from concourse.bass_utils import run_bass_kernel_spmd

    x = np.asarray(x, dtype=np.float32)
    W_qkv = np.asarray(W_qkv, dtype=np.float32)
    b_qkv = np.asarray(b_qkv, dtype=np.float32)
    W_out = np.asarray(W_out, dtype=np.float32)
    b_out = np.asarray(b_out, dtype=np.float32)

    cos128, sin128 = _rope_tables()

    in_maps = []
    for core in range(NCORES):
        b, g = core // G, core % G
        sl = slice(g * DQ, (g + 1) * DQ)
        wqv = W_qkv[:, 0 * D : 1 * D][:, sl]
        wkv = W_qkv[:, 1 * D : 2 * D][:, sl]
        wvv = W_qkv[:, 2 * D : 3 * D][:, sl]
        bqv = b_qkv[0 * D : 1 * D][sl]
        bkv = b_qkv[1 * D : 2 * D][sl]
        bvv = b_qkv[2 * D : 3 * D][sl]
        in_maps.append(
            {
                "xT": _bf16(x[b].T),
                "wq": _bf16(wqv),
                "wk": _bf16(wkv),
                "wv": _bf16(wvv),
                "wo": _bf16(W_out[sl, :]),
                "bq": np.ascontiguousarray(bqv.reshape(PAIRS, 128).T),
                "bk": np.ascontiguousarray(bkv.reshape(PAIRS, 128).T),
                "bv": np.tile(bvv[None, :], (128, 1)).astype(np.float32),
                "cosT": _bf16(cos128),
                "sinT": _bf16(sin128),
                "trin": _bf16(_tri_neg()),
            }
        )

    if _COMPILED is None:
        nc = build_module()
        fixed = legalize_bir_waits(nc.to_json_bytes())
        nc.to_json_bytes = lambda: fixed  # bass2jax ships this BIR to walrus
        _COMPILED = nc
    nc = _COMPILED

    res = run_bass_kernel_spmd(
        nc,
        in_maps,
        core_ids=list(range(NCORES)),
        trace=bool(os.environ.get("BASS_TRACE")),
    )
    LAST_RESULTS = res

    out = np.zeros((B, L, D), dtype=np.float32)
    for core in range(NCORES):
        out[core // G] += np.asarray(res.results[core]["out"], dtype=np.float32)
    out += b_out[None, None, :]
    return out


# revision 38
# speedup vs baseline: 1.0063x; 1.0013x over previous
"""Causal self-attention with RoPE on 8 Trainium2 NeuronCores.

Sharding: batch x head-group. Core c handles batch b = c//2 and head group
g = c%2 (8 of 16 heads). Each core runs the full per-(batch, head-group)
pipeline on device:

  QKV^T projection -> RoPE -> causal flash-style attention -> partial
  output projection (its heads' slice of W_out rows).

The host sums the two partial projections per batch and adds b_out.

v3 pipeline (v2 measured 414us: attention phase ACT-bound at ~95% Scalar
busy while PE idled ~40%, QKV phase the reverse): QKV projection + RoPE
for chunk c+1 are emitted INTO the PE-idle slots between attention pairs
of chunk c (the PE queue is in-order, so placement in program order is
what fills the gaps).  Per-pair boundary schedule for chunk c:
  b0: div-finishB(half1, c-1) | qkv-unit(c+1, mt0) | v-unit(c+1, 0)
  b1: div-finishA(half0, c)   | proj(c-1)          | qkv(c+1, mt1) | v(..1)
  b2: div-finishB(half0, c)   | qkv(c+1, mt2)      | v(..2)
  b3: div-finishA(half1, c)   | qkv(c+1, mt3)      | v(..3)
finishA (the 4us batched reciprocal) always lands one full pair kt-loop
before the finishB that consumes it, and proj(c) one boundary after the
final normalize-mul of chunk c, so no PE-visible dependency ever stalls.

Other changes vs the 572us baseline: both heads of a pair share one
[128,1024] score PSUM tile and ONE wide exp ACTIVATE (amortizes ACT's
352-cycle fixed cost); PV PSUM is drained to SBUF right after stop=True so
the softmax divide chain never blocks bank reuse (the baseline's serialized
4us reciprocals caused 3-4us PE gaps that HAM-throttled the PE to 1.2GHz
for 58% of the runtime); reciprocals are batched 4 denominator rows ->
one DVE op; score/exp/PV all skip the fully-masked columns left of the
causal staircase; the staircase itself is a 0/1 DVE multiply on the
[128,128] window only; QKV biases on DVE (tensor_scalar_add); bf16 output
(halves the 8MB/core output DMA); small SBUF-SBUF moves issue from the
GPSIMD DMA queue so they never queue behind bulk input loads.
"""

import os
import sys

if "/opt/trn_rl_repo" not in sys.path:
    sys.path.insert(0, "/opt/trn_rl_repo")

import numpy as np
import ml_dtypes

import concourse.bass as bass
import concourse.mybir as mybir
import concourse.tile as tile

F32 = mybir.dt.float32
F32R = mybir.dt.float32r
BF16 = mybir.dt.bfloat16

B, L, D = 4, 2048, 1024
H, DH = 16, 64
NCORES = 8
G = 2                 # head groups (cores per batch)
HPC = H // G          # heads per core = 8
DQ = HPC * DH         # per-core q/k/v width = 512
PAIRS = HPC // 2      # 128-partition head pairs = 4
CHUNK = 512           # query-chunk (matmul free dim)
NCH = L // CHUNK      # 4
KT = D // 128         # 8 k-tiles over d_model
LT = L // 128         # 16 l-tiles
VW = DH + 1           # V columns per head incl. ones column = 65

LAST_RESULTS = None   # test harness reads perf fields from here


def legalize_bir_waits(bir_json: bytes) -> bytes:
    """Split multi-wait sync_infos into standalone EventSemaphore instrs.

    This container's walrus codegen accepts at most ONE sync wait per
    instruction (two for EventSemaphore), but Tile's sem assigner happily
    attaches several.  For every instruction carrying N>1 waits, keep one
    and hoist the rest onto EventSemaphore instructions inserted directly
    before it on the same engine (same block), which preserves each
    engine's program order and therefore the sync semantics.
    """
    import json as _json

    j = _json.loads(bir_json)
    uid = [0]
    for fn in j["functions"]:
        for blk in fn["blocks"]:
            out_insts = []
            for inst in blk["instructions"]:
                si = inst.get("sync_info")
                waits = (si or {}).get("on_wait") or []
                cap = 2 if inst.get("opcode") == "EventSemaphore" else 1
                if len(waits) > cap:
                    extra, keep = waits[:-cap], waits[-cap:]
                    for i in range(0, len(extra), 2):
                        uid[0] += 1
                        out_insts.append(
                            {
                                "name": f"antwaitfix-{uid[0]}",
                                "opcode": "EventSemaphore",
                                "engine": inst["engine"],
                                "ins": [],
                                "outs": [],
                                "debug": inst.get("debug", 0),
                                "sync_info": {
                                    "on_wait": extra[i : i + 2],
                                    "on_update": [],
                                },
                            }
                        )
                    si["on_wait"] = keep
                out_insts.append(inst)
            blk["instructions"] = out_insts
    return _json.dumps(j).encode()


def build_module():
    nc = bass.Bass(use_seq_codegen=True)

    xT = nc.declare_dram_parameter("xT", [D, L], BF16, isOutput=False)
    wq = nc.declare_dram_parameter("wq", [D, DQ], BF16, isOutput=False)
    wk = nc.declare_dram_parameter("wk", [D, DQ], BF16, isOutput=False)
    wv = nc.declare_dram_parameter("wv", [D, DQ], BF16, isOutput=False)
    wo = nc.declare_dram_parameter("wo", [DQ, D], BF16, isOutput=False)
    bq = nc.declare_dram_parameter("bq", [128, PAIRS], F32, isOutput=False)
    bk = nc.declare_dram_parameter("bk", [128, PAIRS], F32, isOutput=False)
    bv = nc.declare_dram_parameter("bv", [128, DQ], F32, isOutput=False)
    cosT = nc.declare_dram_parameter("cosT", [128, L], BF16, isOutput=False)
    sinT = nc.declare_dram_parameter("sinT", [128, L], BF16, isOutput=False)
    trin = nc.declare_dram_parameter("trin", [128, 128], BF16, isOutput=False)
    out = nc.declare_dram_parameter("out", [L, D], BF16, isOutput=True)

    with tile.TileContext(nc) as tc:
        with (
            tc.tile_pool(name="const", bufs=1) as cp,
            tc.tile_pool(name="acts", bufs=1) as ap,
            tc.tile_pool(name="work", bufs=2) as wp,
            tc.tile_pool(name="psum", bufs=2, space="PSUM") as ps,
        ):
            # ---- PE warm-up: HAM boots throttled (K=4/8, 1.2GHz) and
            # needs ~3.4us of sustained matmul activity to open the clock
            # gate; the input DMA stream takes ~11us before the first real
            # matmul. Burn the idle time on zero matmuls so the real
            # pipeline starts, and stays, at 2.4GHz.
            warm = cp.tile([128, CHUNK], BF16)
            nc.vector.memset(warm[:], 0.0)
            for i in range(40):
                wps = ps.tile([128, CHUNK], F32, tag="qp", name=f"warm_{i}")
                nc.tensor.matmul(wps[:], warm[:, 0:128], warm[:],
                                 start=True, stop=True)

            # ---- input loads, ordered so the first QKV chain can start
            # ~1.3us in: per-kt rounds of (wq, wk, xT chunk-0), then wv and
            # the rest of xT chunk by chunk.
            xT_sb = ap.tile([128, KT, L], BF16)
            wq_sb = cp.tile([128, KT, DQ], BF16)
            wk_sb = cp.tile([128, KT, DQ], BF16)
            wv_sb = cp.tile([128, KT, DQ], BF16)
            xT_r = xT.rearrange("(kt p) l -> p kt l", p=128)
            # first k-tile round fans out over three engine DMA queues so
            # the first QKV matmul's inputs land in parallel
            wq_r = wq.rearrange("(kt p) m -> p kt m", p=128)
            wk_r = wk.rearrange("(kt p) m -> p kt m", p=128)
            wv_r = wv.rearrange("(kt p) m -> p kt m", p=128)
            for kt in range(KT):
                nc.sync.dma_start(wq_sb[:, kt, :], wq_r[:, kt, :])
                (nc.gpsimd if kt < 2 else nc.sync).dma_start(
                    wk_sb[:, kt, :], wk_r[:, kt, :]
                )
                (nc.scalar if kt < 2 else nc.sync).dma_start(
                    xT_sb[:, kt, 0:CHUNK], xT_r[:, kt, 0:CHUNK]
                )
                nc.sync.dma_start(wv_sb[:, kt, :], wv_r[:, kt, :])
            bq_sb = cp.tile([128, PAIRS], F32)
            bk_sb = cp.tile([128, PAIRS], F32)
            bv_sb = cp.tile([128, DQ], F32)
            cos_sb = cp.tile([128, L], BF16)
            sin_sb = cp.tile([128, L], BF16)
            tri_sb = cp.tile([128, 128], BF16)
            nc.sync.dma_start(cos_sb[:], cosT[:])
            nc.sync.dma_start(sin_sb[:], sinT[:])
            nc.sync.dma_start(bq_sb[:], bq[:])
            nc.sync.dma_start(bk_sb[:], bk[:])
            nc.sync.dma_start(bv_sb[:], bv[:])
            nc.sync.dma_start(tri_sb[:], trin[:])
            # rest of xT streams in while chunk-0 computes; wo last (first
            # needed by proj(0) at ~55us).  These stay on the sync queue —
            # the gpsimd queue must stay clear for the latency-critical
            # RoPE-swap / den-stage moves.
            for kt in range(KT):
                nc.sync.dma_start(
                    xT_sb[:, kt, CHUNK:L], xT_r[:, kt, CHUNK:L]
                )
            wo_sb = cp.tile([128, PAIRS, D], BF16)
            for pr in range(PAIRS):
                nc.sync.dma_start(
                    wo_sb[:, pr, :], wo.rearrange("(pr p) c -> p pr c", p=128)[:, pr, :]
                )
            # bf16 ones/reciprocals: the K=1 broadcast matmul then runs in
            # bf16 (fp32r forces the PE's slow HIGH mode, ~4x the cycles);
            # denominators at bf16 cost ~0.4% relative, well inside the
            # tolerance budget.
            ones_sb = cp.tile([128, 64], BF16)
            nc.vector.memset(ones_sb[:], 1.0)

            qT_sb = ap.tile([128, PAIRS, L], BF16)
            kT_sb = ap.tile([128, PAIRS, L], BF16)
            v_sb = ap.tile([128, LT, HPC * VW], BF16)
            yT_sb = ap.tile([128, PAIRS, L], BF16)

            # ---------------- emission units ----------------
            def qkv_unit(c, mt):
                """q+k projection, bias, and RoPE for (pair mt, chunk c)."""
                q0 = c * CHUNK
                qk = ps.tile([128, 1024], F32, tag="sc", name=f"qk_{c}_{mt}")
                for half, w_sb in ((0, wq_sb), (1, wk_sb)):
                    for kt in range(KT):
                        nc.tensor.matmul(
                            qk[:, half * 512 : half * 512 + 512],
                            w_sb[:, kt, mt * 128 : (mt + 1) * 128],
                            xT_sb[:, kt, q0 : q0 + CHUNK],
                            start=(kt == 0),
                            stop=(kt == KT - 1),
                        )
                nc.vector.tensor_scalar_add(
                    qT_sb[:, mt, q0 : q0 + CHUNK], qk[:, 0:512],
                    bq_sb[:, mt : mt + 1],
                )
                nc.vector.tensor_scalar_add(
                    kT_sb[:, mt, q0 : q0 + CHUNK], qk[:, 512:1024],
                    bk_sb[:, mt : mt + 1],
                )
                # RoPE in place on this (pair, chunk) slice; the rotate-half
                # partition swaps run on the idle GPSIMD engine (it has no
                # partition-base-match restriction), keeping SP/DVE free.
                for dst in (qT_sb, kT_sb):
                    t = dst[:, mt, q0 : q0 + CHUNK]
                    swp = wp.tile([128, CHUNK], BF16, tag="swp", bufs=2)
                    for i in range(4):
                        j = i ^ 1
                        nc.gpsimd.dma_start(
                            swp[i * 32 : (i + 1) * 32, :],
                            t[j * 32 : (j + 1) * 32, :],
                        )
                    nc.vector.tensor_mul(
                        swp[:], swp[:], sin_sb[:, q0 : q0 + CHUNK]
                    )
                    nc.vector.tensor_mul(t, t, cos_sb[:, q0 : q0 + CHUNK])
                    nc.vector.tensor_add(t, t, swp[:])

            def v_unit(lt):
                """V projection (+bias, ones column) for l-tile lt."""
                vps = ps.tile([128, CHUNK], F32, tag="qp", name=f"vps_{lt}")
                for kt in range(KT):
                    nc.tensor.matmul(
                        vps[:],
                        xT_sb[:, kt, lt * 128 : (lt + 1) * 128],
                        wv_sb[:, kt, :],
                        start=(kt == 0),
                        stop=(kt == KT - 1),
                    )
                vdst = v_sb[:, lt, :].rearrange("p (h c) -> p h c", c=VW)
                nc.vector.tensor_add(vdst[:, :, 0:DH], vps[:], bv_sb[:])
                nc.vector.memset(vdst[:, :, DH:VW], 1.0)

            def make_proj(c):
                def proj():
                    for lt in range(4 * c, 4 * c + 4):
                        for cc in range(2):
                            op = ps.tile([128, CHUNK], F32, tag="qp",
                                         name=f"op_{lt}_{cc}")
                            for pr in range(PAIRS):
                                nc.tensor.matmul(
                                    op[:],
                                    yT_sb[:, pr, lt * 128 : (lt + 1) * 128],
                                    wo_sb[:, pr, cc * CHUNK : (cc + 1) * CHUNK],
                                    start=(pr == 0),
                                    stop=(pr == PAIRS - 1),
                                )
                            ob = wp.tile([128, CHUNK], BF16, tag="ob", bufs=2)
                            nc.vector.tensor_copy(ob[:], op[:])
                            nc.sync.dma_start(
                                out[
                                    lt * 128 : (lt + 1) * 128,
                                    cc * CHUNK : (cc + 1) * CHUNK,
                                ],
                                ob[:],
                            )
                return proj

            # ---- pipeline prologue: chunk-0 QKV kt-OUTER across all 4
            # pairs (8 MMs per arriving xT k-tile -> full PE duty while the
            # input streams in).  mt 0/1 use the two [128,1024] sc tiles,
            # mt 2/3 split their q/k chains across the ys/qp rings — the
            # whole 8-bank PSUM is otherwise idle during the prologue.
            qk01 = [ps.tile([128, 1024], F32, tag="sc", name=f"qk0_{m}")
                    for m in (0, 1)]
            q2 = ps.tile([128, CHUNK], F32, tag="ys", name="q2_pro")
            k2 = ps.tile([128, CHUNK], F32, tag="ys", name="k2_pro")
            q3 = ps.tile([128, CHUNK], F32, tag="qp", name="q3_pro")
            k3 = ps.tile([128, CHUNK], F32, tag="qp", name="k3_pro")
            pro = {
                (0, 0): qk01[0][:, 0:512], (0, 1): qk01[0][:, 512:1024],
                (1, 0): qk01[1][:, 0:512], (1, 1): qk01[1][:, 512:1024],
                (2, 0): q2[:], (2, 1): k2[:],
                (3, 0): q3[:], (3, 1): k3[:],
            }
            for kt in range(KT):
                for mt in range(PAIRS):
                    for half, w_sb in ((0, wq_sb), (1, wk_sb)):
                        nc.tensor.matmul(
                            pro[(mt, half)],
                            w_sb[:, kt, mt * 128 : (mt + 1) * 128],
                            xT_sb[:, kt, 0:CHUNK],
                            start=(kt == 0),
                            stop=(kt == KT - 1),
                        )
            for lt in range(4):
                v_unit(lt)
            for mt in range(PAIRS):
                nc.vector.tensor_scalar_add(
                    qT_sb[:, mt, 0:CHUNK], pro[(mt, 0)], bq_sb[:, mt : mt + 1]
                )
                nc.vector.tensor_scalar_add(
                    kT_sb[:, mt, 0:CHUNK], pro[(mt, 1)], bk_sb[:, mt : mt + 1]
                )
                for dst in (qT_sb, kT_sb):
                    t = dst[:, mt, 0:CHUNK]
                    swp = wp.tile([128, CHUNK], BF16, tag="swp", bufs=2)
                    for i in range(4):
                        j = i ^ 1
                        nc.gpsimd.dma_start(
                            swp[i * 32 : (i + 1) * 32, :],
                            t[j * 32 : (j + 1) * 32, :],
                        )
                    nc.vector.tensor_mul(swp[:], swp[:], sin_sb[:, 0:CHUNK])
                    nc.vector.tensor_mul(t, t, cos_sb[:, 0:CHUNK])
                    nc.vector.tensor_add(t, t, swp[:])

            # ---- attention pipeline
            pending = []           # staggered div-finish / proj closures

            def drain_pending(n):
                for _ in range(min(n, len(pending))):
                    pending.pop(0)()

            for c in range(NCH):
                q0 = c * CHUNK
                n_lk = 4 * (c + 1)
                half_state = {}
                for pr in range(PAIRS):
                    ys = [
                        ps.tile([128, CHUNK], F32, tag="ys",
                                name=f"ys_{c}_{pr}_{hh}")
                        for hh in range(2)
                    ]
                    for kt in range(n_lk):
                        k0 = kt * 128
                        off = max(0, k0 - q0)
                        sps = ps.tile([128, 1024], F32, tag="sc",
                                      name=f"sps_{c}_{pr}_{kt}")
                        diag = k0 >= q0
                        for hh in range(2):
                            # columns left of the staircase are fully masked
                            # and never read (exp and PV skip them too), so
                            # the score matmul starts at `off`.
                            nc.tensor.matmul(
                                sps[:, hh * 512 + off : hh * 512 + 512],
                                kT_sb[hh * 64 : (hh + 1) * 64, pr, k0 : k0 + 128],
                                qT_sb[hh * 64 : (hh + 1) * 64, pr,
                                      q0 + off : q0 + CHUNK],
                                start=True,
                                stop=True,
                                tile_position=(hh * 64, 0),
                            )
                        ex = wp.tile([128, 1024], BF16, tag="ex", bufs=4)
                        if off:
                            # skip the fully-masked columns left of the
                            # staircase (strided 2-window AP)
                            nc.scalar.activation(
                                ex.rearrange("p (h w) -> p h w", w=512)[:, :, off:],
                                sps.rearrange("p (h w) -> p h w", w=512)[:, :, off:],
                                mybir.ActivationFunctionType.Exp,
                                scale=float(1.0 / np.sqrt(DH)),
                            )
                        else:
                            nc.scalar.activation(
                                ex[:], sps[:], mybir.ActivationFunctionType.Exp,
                                scale=float(1.0 / np.sqrt(DH)),
                            )
                        if diag:
                            for hh in range(2):
                                nc.vector.tensor_mul(
                                    ex[:, hh * 512 + off : hh * 512 + off + 128],
                                    ex[:, hh * 512 + off : hh * 512 + off + 128],
                                    tri_sb[:],
                                )
                        for hh in range(2):
                            h = 2 * pr + hh
                            nc.tensor.matmul(
                                ys[hh][0:VW, off:CHUNK],
                                v_sb[:, kt, h * VW : (h + 1) * VW],
                                ex[:, hh * 512 + off : (hh + 1) * 512],
                                start=(kt == 0),
                                stop=(kt == n_lk - 1),
                            )
                    # drain PV psum to SBUF fast (frees banks; PE rolls on)
                    if pr % 2 == 0:
                        stage = wp.tile([128, 128], F32, tag="stage", bufs=2,
                                        name=f"stage_{c}_{pr // 2}")
                        half_state = {"stage": stage, "tiles": []}
                    for hh in range(2):
                        ya = wp.tile([65, CHUNK], F32, tag="ya", bufs=10,
                                     name=f"ya_{c}_{pr}_{hh}")
                        nc.vector.tensor_copy(ya[:], ys[hh][0:VW, :])
                        idx = (pr % 2) * 2 + hh
                        # pack the [1,512] denominator row as [4,128] across
                        # partitions so the reciprocal runs 4 lanes wide
                        nc.gpsimd.dma_start(
                            half_state["stage"][32 * idx : 32 * idx + 4, :],
                            ya[64:65, :].rearrange("p (a j) -> p a j", a=4),
                        )
                        half_state["tiles"].append((pr, hh, ya, idx))
                    if pr % 2 == 1:
                        st = half_state
                        stash = {}

                        def finishA(st=st, stash=stash):
                            stage_r = wp.tile([128, 128], BF16, tag="str",
                                              bufs=2)
                            with nc.allow_low_precision(reason="bf16 recip"):
                                nc.vector.reciprocal(stage_r[:], st["stage"][:])
                            den4 = wp.tile([128, CHUNK], BF16, tag="den4",
                                           bufs=2)
                            for _, _, _, idx in st["tiles"]:
                                nc.gpsimd.dma_start(
                                    den4[32 * idx : 32 * idx + 1, :],
                                    stage_r[32 * idx : 32 * idx + 4, :],
                                )
                            stash["den4"] = den4

                        def finishB(st=st, stash=stash, q0=q0):
                            den4 = stash["den4"]
                            for pr2, hh, ya, idx in st["tiles"]:
                                bc = ps.tile([128, CHUNK], F32, tag="qp",
                                             name=f"bc_{q0}_{pr2}_{hh}")
                                nc.tensor.matmul(
                                    bc[0:64, :],
                                    ones_sb[32 * idx : 32 * idx + 1, :],
                                    den4[32 * idx : 32 * idx + 1, :],
                                    start=True,
                                    stop=True,
                                    tile_position=(32 * idx, 0),
                                )
                                if hh == 0:
                                    nc.vector.tensor_mul(
                                        yT_sb[0:64, pr2, q0 : q0 + CHUNK],
                                        ya[0:64, :],
                                        bc[0:64, :],
                                    )
                                else:
                                    # elementwise out/in partition bases must
                                    # match; base-0 tmp + DMA moves to 64:128.
                                    yt = wp.tile([64, CHUNK], BF16, tag="yt",
                                                 bufs=2)
                                    nc.vector.tensor_mul(
                                        yt[:], ya[0:64, :], bc[0:64, :]
                                    )
                                    nc.gpsimd.dma_start(
                                        yT_sb[64:128, pr2, q0 : q0 + CHUNK],
                                        yt[:],
                                    )

                        pending.append(finishA)
                        pending.append(finishB)
                    # ---- pair boundary: next chunk's QKV/V first (their DVE
                    # bias/RoPE ops gate a later pair's score matmuls, so
                    # they must beat the div-chain's reciprocal into the DVE
                    # queue), then the staggered div/proj items.  Drains must
                    # total 5 per chunk (2 finishes x2 + proj) or the queue
                    # drifts and the tail serializes; the double-drain sits
                    # at p1 (proj(c-1) then finishA) where both items'
                    # dependencies are a full pair-loop old.
                    if c + 1 < NCH:
                        qkv_unit(c + 1, pr)
                        v_unit(4 * (c + 1) + pr)
                    drain_pending(2 if (pr == 1 and c > 0) else 1)

                pending.append(make_proj(c))
            drain_pending(len(pending))
    return nc


def _rope_tables():
    inv_freq = (1.0 / (10000.0 ** (np.arange(0, DH, 2, dtype=np.float32) / DH))).astype(
        np.float32
    )
    t = np.arange(L, dtype=np.float32)
    freqs = np.einsum("l,d->ld", t, inv_freq).astype(np.float32)  # (L, 32)
    emb = np.concatenate([freqs, freqs], axis=-1)                 # (L, 64)
    cos = np.cos(emb).astype(np.float32)
    sin = np.sin(emb).astype(np.float32)
    cosT = cos.T                                   # (64, L)
    sinT = sin.T.copy()
    sinT[0:32] = -sinT[0:32]                       # fold rotate_half sign
    cos128 = np.tile(cosT, (2, 1))                 # (128, L)
    sin128 = np.tile(sinT, (2, 1))
    return cos128, sin128


def _tri_neg():
    # trin[p, t] = 0 where key-partition p is masked for query-col t of the
    # staircase window (p > t), else 1 (multiplicative mask on exp scores).
    p = np.arange(128)[:, None]
    t = np.arange(128)[None, :]
    return (p <= t).astype(np.float32)


def _bf16(a):
    return np.asarray(a, dtype=np.float32).astype(ml_dtypes.bfloat16)


_COMPILED = None


def _ensure_ntff_hook():
    """This image's antenv lacks axon_hooks; trace=True would crash on the
    import inside run_bass_kernel_spmd. Inject the module and register the
    ctypes NTFF hook so tracing works; silently skip if anything is off."""
    import types

    if "antenv.axon_hooks" in sys.modules:
        return
    try:
        mod = types.ModuleType("antenv.axon_hooks")
        store = [None]
        mod.set_axon_ntff_profile_hook = lambda h: store.__setitem__(0, h)
        mod.get_axon_ntff_profile_hook = lambda: store[0]
        from trn_agent_boot.trn_boot import _ntff_profile_via_ctypes

        mod.set_axon_ntff_profile_hook(
            _ntff_profile_via_ctypes("/opt/axon/libaxon_pjrt.so")
        )
        sys.modules["antenv.axon_hooks"] = mod
    except Exception:
        pass


def kernel(x, pad_mask, W_qkv, b_qkv, W_out, b_out):
    global LAST_RESULTS, _COMPILED
    if os.environ.get("BASS_TRACE"):
        _ensure_ntff_hook()
    from concourse.bass_utils import run_bass_kernel_spmd

    x = np.asarray(x, dtype=np.float32)
    W_qkv = np.asarray(W_qkv, dtype=np.float32)
    b_qkv = np.asarray(b_qkv, dtype=np.float32)
    W_out = np.asarray(W_out, dtype=np.float32)
    b_out = np.asarray(b_out, dtype=np.float32)

    cos128, sin128 = _rope_tables()

    in_maps = []
    for core in range(NCORES):
        b, g = core // G, core % G
        sl = slice(g * DQ, (g + 1) * DQ)
        wqv = W_qkv[:, 0 * D : 1 * D][:, sl]
        wkv = W_qkv[:, 1 * D : 2 * D][:, sl]
        wvv = W_qkv[:, 2 * D : 3 * D][:, sl]
        bqv = b_qkv[0 * D : 1 * D][sl]
        bkv = b_qkv[1 * D : 2 * D][sl]
        bvv = b_qkv[2 * D : 3 * D][sl]
        in_maps.append(
            {
                "xT": _bf16(x[b].T),
                "wq": _bf16(wqv),
                "wk": _bf16(wkv),
                "wv": _bf16(wvv),
                "wo": _bf16(W_out[sl, :]),
                "bq": np.ascontiguousarray(bqv.reshape(PAIRS, 128).T),
                "bk": np.ascontiguousarray(bkv.reshape(PAIRS, 128).T),
                "bv": np.tile(bvv[None, :], (128, 1)).astype(np.float32),
                "cosT": _bf16(cos128),
                "sinT": _bf16(sin128),
                "trin": _bf16(_tri_neg()),
            }
        )

    if _COMPILED is None:
        nc = build_module()
        fixed = legalize_bir_waits(nc.to_json_bytes())
        nc.to_json_bytes = lambda: fixed  # bass2jax ships this BIR to walrus
        _COMPILED = nc
    nc = _COMPILED

    res = run_bass_kernel_spmd(
        nc,
        in_maps,
        core_ids=list(range(NCORES)),
        trace=bool(os.environ.get("BASS_TRACE")),
    )
    LAST_RESULTS = res

    out = np.zeros((B, L, D), dtype=np.float32)
    for core in range(NCORES):
        out[core // G] += np.asarray(res.results[core]["out"], dtype=np.float32)
    out += b_out[None, None, :]
    return out
